# revision 18
# baseline (speedup 1.0000x reference)
"""AdaFace-style margin-softmax loss head, distributed over 8 TRN2 NeuronCores.

Math (see problem reference):
    kernel_norm = kernel / ||kernel||_col
    cosine = clip(emb @ kernel_norm, -1+eps, 1-eps)            [B, C]
    margin_scaler = clip((safe_norms - mean)/(std+eps) * H, -1, 1)
    g_ang = -M * margin_scaler ;  g_add = M + M*margin_scaler
    theta_m = clip(arccos(cosine) + onehot_label*g_ang, eps, pi-eps)
    cos_m = cos(theta_m);  am = argmax(cos_m, 1)
    cos_max = cos(clip(theta_m + onehot_am*g_ang, eps, pi-eps))
    out1 = (cos_m - onehot_label*g_add)*S
    out2 = (cos_max - onehot_am*g_add)*S
    returns (out1, out2, am, margin_scaler)

Key structural facts exploited:
  * out1 == S*cosine except at the label column of each row.
  * out2 == S*cosine except at the label column and the argmax column.
  * cos(arccos(c)+g) = c*cos(g) - sqrt(1-c^2)*sin(g)  -> no arccos needed.
  * argmax(cos_m) needs fp32-accuracy matmul (bf16 noise flips it), so the
    big matmul runs in native fp32 on the PE.

Sharding: kernel and the [B, C] outputs are split along the class dim into
8 shards of 8844 columns (last 30 cols of core 7 are padding, masked to -1e4
before the argmax and dropped on the host). The label-column cosines are
computed on every core from a replicated gathered kernel[:, label] input,
so no device-side gathers are needed. One AllGather of per-core
(max, argmax) candidate pairs resolves the global argmax.
"""

import math
import os
from contextlib import ExitStack

import numpy as np

import concourse.bass as bass
import concourse.mybir as mybir
import concourse.tile as tile
from concourse import bacc
from concourse.bass_utils import run_bass_kernel_spmd
from concourse.masks import make_identity

F32 = mybir.dt.float32
U32 = mybir.dt.uint32
AF = mybir.ActivationFunctionType
OP = mybir.AluOpType

B = 512          # batch
K = 512          # embedding dim
C = 70722        # number of classes
NCORES = 8
CSH = 8844       # per-core class shard (8*8844 = 70752 = C + 30 pad)
NPAD = NCORES * CSH - C  # 30
P = 128          # partitions
NT = B // P      # 4 row tiles (also 4 k tiles since K == B)
CT = [512] * 17 + [CSH - 17 * 512]   # c-tile widths inside a shard
# c-tile index ranges forming argmax chunks (partial top-8 scans overlap PE)
CHUNKS = [(0, 6), (6, 12), (12, 18)]

M_CONST = 0.4
H_CONST = 0.333
S_CONST = 64.0
EPS = 1e-3
COS_HI = float(np.float32(math.cos(EPS)))         # cos(eps)
COS_LO = float(np.float32(math.cos(math.pi - EPS)))  # cos(pi-eps)
NEG_BIG = -1.0e9
POS_BIG = 1.0e9
PAD_MASK_VAL = -1.0e4


def build_nc():
    nc = bacc.Bacc(None, num_devices=NCORES)

    embT = nc.declare_dram_parameter("embT", [K, B], F32, isOutput=False)
    ksh = nc.declare_dram_parameter("ksh", [K, CSH], F32, isOutput=False)
    klab = nc.declare_dram_parameter("klab", [K, B], F32, isOutput=False)
    normsv = nc.declare_dram_parameter("normsv", [B], F32, isOutput=False)
    labloc = nc.declare_dram_parameter("labloc", [B], F32, isOutput=False)
    labmask = nc.declare_dram_parameter("labmask", [B], F32, isOutput=False)
    labglob = nc.declare_dram_parameter("labglob", [B], F32, isOutput=False)
    coloff = nc.declare_dram_parameter("coloff", [B], F32, isOutput=False)
    padmask = nc.declare_dram_parameter("padmask", [NPAD], F32, isOutput=False)

    o1 = nc.declare_dram_parameter("o1", [B, CSH], F32, isOutput=True)
    o2 = nc.declare_dram_parameter("o2", [B, CSH], F32, isOutput=True)
    amax = nc.declare_dram_parameter("amax", [B], F32, isOutput=True)
    marg = nc.declare_dram_parameter("marg", [B], F32, isOutput=True)
    fix1 = nc.declare_dram_parameter("fix1", [B], F32, isOutput=True)
    fix2l = nc.declare_dram_parameter("fix2l", [B], F32, isOutput=True)
    fix2a = nc.declare_dram_parameter("fix2a", [B], F32, isOutput=True)

    # internal DRAM bounce buffers for the collective
    cand_dram = nc.dram_tensor("cand_bounce", [B, 2], F32)
    gath_dram = nc.dram_tensor("gath_bounce", [NCORES * B, 2], F32,
                               addr_space="Shared")

    with ExitStack() as ctx:
        tc = ctx.enter_context(tile.TileContext(nc, num_cores=NCORES))

        persist = ctx.enter_context(tc.tile_pool(name="persist", bufs=1))
        small = ctx.enter_context(tc.tile_pool(name="small", bufs=1))
        psum_mm = ctx.enter_context(tc.tile_pool(name="psum_mm", bufs=2, space="PSUM"))
        psum_n = ctx.enter_context(tc.tile_pool(name="psum_n", bufs=2, space="PSUM"))
        psum_misc = ctx.enter_context(tc.tile_pool(name="psum_misc", bufs=1, space="PSUM"))
        psum_bc = ctx.enter_context(tc.tile_pool(name="psum_bc", bufs=2, space="PSUM"))

        # --------- resident tiles ---------
        # S*cosine for the whole shard: [p, row-tile, col]
        out_sb = persist.tile([P, NT, CSH], F32)
        # embT as 4 k-subtiles side by side: [p, ktile, bcol]
        embT_sb = persist.tile([P, NT, B], F32)
        nc.sync.dma_start(out=embT_sb,
                          in_=embT.ap().rearrange("(kt p) b -> p kt b", p=P))

        ones_col = small.tile([P, 1], F32)
        nc.vector.memset(ones_col, 1.0)
        ones_row = small.tile([1, P], F32)
        nc.vector.memset(ones_row, 1.0)
        ident_sb = small.tile([P, P], F32)
        make_identity(nc, ident_sb)
        bigpos8 = small.tile([P, 8], F32)
        nc.vector.memset(bigpos8, POS_BIG)
        negbig1 = small.tile([P, 1], F32)
        nc.vector.memset(negbig1, NEG_BIG)
        padmask_sb = small.tile([P, NPAD], F32)
        nc.sync.dma_start(out=padmask_sb,
                          in_=bass.AP(tensor=padmask, offset=0,
                                      ap=[[0, P], [1, NPAD]]))

        def load_rowvec(dram_h, tag):
            t = small.tile([P, NT], F32, tag=tag)
            nc.sync.dma_start(out=t,
                              in_=dram_h.ap().rearrange("(t p) -> p t", p=P))
            return t

        norms_sb = load_rowvec(normsv, "norms_sb")
        labloc_sb = load_rowvec(labloc, "labloc_sb")
        labmask_sb = load_rowvec(labmask, "labmask_sb")
        labglob_sb = load_rowvec(labglob, "labglob_sb")
        coloff_sb = load_rowvec(coloff, "coloff_sb")
        labmask_i = small.tile([P, NT], mybir.dt.int32, tag="labmask_i")
        nc.vector.tensor_copy(out=labmask_i, in_=labmask_sb)

        # --------- margin pipeline (tiny) ---------
        sn = small.tile([P, NT], F32)
        nc.vector.tensor_scalar(out=sn, in0=norms_sb, scalar1=0.001, scalar2=100.0,
                                op0=OP.max, op1=OP.min)
        sn_sq = small.tile([P, NT], F32)
        nc.vector.tensor_tensor(out=sn_sq, in0=sn, in1=sn, op=OP.mult)
        stats_ps = psum_misc.tile([1, 2 * NT], F32)
        nc.tensor.matmul(stats_ps[:, 0:NT], lhsT=ones_col, rhs=sn,
                         start=True, stop=True)
        nc.tensor.matmul(stats_ps[:, NT:2 * NT], lhsT=ones_col, rhs=sn_sq,
                         start=True, stop=True)
        stats_sb = small.tile([1, 2 * NT], F32)
        nc.vector.tensor_copy(out=stats_sb, in_=stats_ps)
        s1 = small.tile([1, 1], F32)
        s2 = small.tile([1, 1], F32)
        nc.vector.reduce_sum(out=s1, in_=stats_sb[:, 0:NT], axis=mybir.AxisListType.X)
        nc.vector.reduce_sum(out=s2, in_=stats_sb[:, NT:2 * NT], axis=mybir.AxisListType.X)
        mean11 = small.tile([1, 1], F32)
        nc.vector.tensor_scalar(out=mean11, in0=s1, scalar1=1.0 / B, scalar2=None,
                                op0=OP.mult)
        # var = (s2 - B*mean^2) / (B-1)
        m2 = small.tile([1, 1], F32)
        nc.vector.tensor_tensor(out=m2, in0=mean11, in1=mean11, op=OP.mult)
        var11 = small.tile([1, 1], F32)
        nc.vector.tensor_scalar(out=m2, in0=m2, scalar1=-float(B), scalar2=None,
                                op0=OP.mult)
        nc.vector.tensor_tensor(out=var11, in0=s2, in1=m2, op=OP.add)
        nc.vector.tensor_scalar(out=var11, in0=var11, scalar1=1.0 / (B - 1),
                                scalar2=None, op0=OP.mult)
        std11 = small.tile([1, 1], F32)
        nc.scalar.activation(out=std11, in_=var11, func=AF.Sqrt)
        # r = 1/(std+eps)
        nc.vector.tensor_scalar(out=std11, in0=std11, scalar1=EPS, scalar2=None,
                                op0=OP.add)
        r11 = small.tile([1, 1], F32)
        nc.vector.reciprocal(out=r11, in_=std11)
        # broadcast (mean, r) across partitions via a K=1 ones-matmul
        mr11 = small.tile([1, 2], F32)
        nc.vector.tensor_copy(out=mr11[:, 0:1], in_=mean11)
        nc.vector.tensor_copy(out=mr11[:, 1:2], in_=r11)
        mr_ps = psum_bc.tile([P, 2], F32, tag="bc")
        nc.tensor.matmul(mr_ps, lhsT=ones_row, rhs=mr11, start=True, stop=True)
        mr_b = small.tile([P, 2], F32)
        nc.vector.tensor_copy(out=mr_b, in_=mr_ps)
        mean_b = mr_b[:, 0:1]
        r_b = mr_b[:, 1:2]
        ms_sb = small.tile([P, NT], F32)
        nc.vector.tensor_scalar(out=ms_sb, in0=sn, scalar1=mean_b, scalar2=None,
                                op0=OP.subtract)
        nc.vector.tensor_scalar(out=ms_sb, in0=ms_sb, scalar1=r_b,
                                scalar2=H_CONST, op0=OP.mult, op1=OP.mult)
        nc.vector.tensor_scalar(out=ms_sb, in0=ms_sb, scalar1=-1.0, scalar2=1.0,
                                op0=OP.max, op1=OP.min)
        ms_out = small.tile([P, NT], F32, tag="ms_out")
        nc.vector.tensor_copy(out=ms_out, in_=ms_sb)
        nc.sync.dma_start(out=marg.ap(), in_=ms_out)

        gadd_sb = small.tile([P, NT], F32)
        nc.vector.tensor_scalar(out=gadd_sb, in0=ms_sb, scalar1=M_CONST,
                                scalar2=M_CONST, op0=OP.mult, op1=OP.add)
        sing_sb = small.tile([P, NT], F32)
        cosg_sb = small.tile([P, NT], F32)
        halfpi_b = small.tile([P, 1], F32)
        nc.vector.memset(halfpi_b, math.pi / 2)
        # g = -M*ms ; sin(g), cos(g)=sin(g + pi/2); activation: func(in*scale+bias)
        nc.scalar.activation(out=sing_sb, in_=ms_sb, func=AF.Sin, scale=-M_CONST)
        nc.scalar.activation(out=cosg_sb, in_=ms_sb, func=AF.Sin, scale=-M_CONST,
                             bias=halfpi_b)

        # --------- klab prologue: label-column cosines for every row ---------
        cl_sb = small.tile([P, NT], F32)
        with tc.tile_pool(name="prolog", bufs=2) as prolog:
            klab_sb = persist.tile([P, NT, B], F32)
            nc.sync.dma_start(out=klab_sb,
                              in_=klab.ap().rearrange("(kt p) b -> p kt b", p=P))
            kn_ps = psum_n.tile([1, B], F32, tag="nps")
            for kt in range(NT):
                sq = prolog.tile([P, B], F32, tag="sq")
                nc.scalar.activation(out=sq, in_=klab_sb[:, kt, :], func=AF.Square)
                nc.tensor.matmul(kn_ps, lhsT=ones_col, rhs=sq,
                                 start=(kt == 0), stop=(kt == NT - 1))
            kn_sb = prolog.tile([1, B], F32, tag="kn")
            nc.scalar.activation(out=kn_sb, in_=kn_ps, func=AF.Sqrt)
            invl = prolog.tile([1, B], F32, tag="invl")
            nc.vector.reciprocal(out=invl, in_=kn_sb)
            invl_ps = psum_bc.tile([P, B], F32, tag="bc")
            nc.tensor.matmul(invl_ps, lhsT=ones_row, rhs=invl, start=True, stop=True)
            for kt in range(NT):
                nc.vector.tensor_tensor(out=klab_sb[:, kt, :], in0=klab_sb[:, kt, :],
                                        in1=invl_ps, op=OP.mult)
            for t in range(NT):
                lab_ps = psum_misc.tile([P, P], F32, tag="labps")
                for kt in range(NT):
                    nc.tensor.matmul(lab_ps,
                                     lhsT=embT_sb[:, kt, t * P:(t + 1) * P],
                                     rhs=klab_sb[:, kt, t * P:(t + 1) * P],
                                     start=(kt == 0), stop=(kt == NT - 1))
                diag = prolog.tile([P, P], F32, tag="diag")
                nc.vector.tensor_tensor(out=diag, in0=lab_ps, in1=ident_sb,
                                        op=OP.mult)
                nc.vector.reduce_sum(out=cl_sb[:, t:t + 1], in_=diag,
                                     axis=mybir.AxisListType.X)
        # clip c_l like the reference clips cosine
        nc.vector.tensor_scalar(out=cl_sb, in0=cl_sb, scalar1=-1.0 + EPS,
                                scalar2=1.0 - EPS, op0=OP.max, op1=OP.min)
        # s_l = sqrt(max(1-c^2, 0))
        sl_sb = small.tile([P, NT], F32)
        nc.vector.tensor_tensor(out=sl_sb, in0=cl_sb, in1=cl_sb, op=OP.mult)
        nc.vector.tensor_scalar(out=sl_sb, in0=sl_sb, scalar1=-1.0, scalar2=1.0,
                                op0=OP.mult, op1=OP.add)
        nc.vector.tensor_scalar(out=sl_sb, in0=sl_sb, scalar1=0.0, scalar2=None,
                                op0=OP.max)
        nc.scalar.activation(out=sl_sb, in_=sl_sb, func=AF.Sqrt)
        # cos_m_l = clip2(c*cosg - s*sing)
        cosml = small.tile([P, NT], F32)
        tmp_b = small.tile([P, NT], F32)
        nc.vector.tensor_tensor(out=cosml, in0=cl_sb, in1=cosg_sb, op=OP.mult)
        nc.vector.tensor_tensor(out=tmp_b, in0=sl_sb, in1=sing_sb, op=OP.mult)
        nc.vector.tensor_tensor(out=cosml, in0=cosml, in1=tmp_b, op=OP.subtract)
        nc.vector.tensor_scalar(out=cosml, in0=cosml, scalar1=COS_LO,
                                scalar2=COS_HI, op0=OP.max, op1=OP.min)
        # fix1 = S*(cos_m_l - g_add) ; fix2l = S*cos_m_l ; clabS = S*cos_m_l
        fix1_sb = small.tile([P, NT], F32)
        nc.vector.tensor_tensor(out=fix1_sb, in0=cosml, in1=gadd_sb, op=OP.subtract)
        nc.vector.tensor_scalar(out=fix1_sb, in0=fix1_sb, scalar1=S_CONST,
                                scalar2=None, op0=OP.mult)
        clabS = small.tile([P, NT], F32)
        nc.vector.tensor_scalar(out=clabS, in0=cosml, scalar1=S_CONST,
                                scalar2=None, op0=OP.mult)
        nc.sync.dma_start(out=fix1.ap(), in_=fix1_sb)
        nc.sync.dma_start(out=fix2l.ap(), in_=clabS)

        # --------- main loop over c-tiles ---------
        mx_sb = small.tile([P, NT, len(CHUNKS), 8], F32)
        idx_sb = small.tile([P, NT, len(CHUNKS), 8], U32)
        with tc.tile_pool(name="mainl", bufs=2) as mainl, \
             tc.tile_pool(name="sqp", bufs=2) as sqp:
            c0 = 0
            for ci, cw in enumerate(CT):
                ksb = mainl.tile([P, NT, cw], F32, tag="ksb")
                nc.sync.dma_start(
                    out=ksb,
                    in_=ksh.ap()[:, c0:c0 + cw].rearrange("(kt p) c -> p kt c", p=P))
                # column sums of squares
                s01 = sqp.tile([P, cw], F32, tag="s01")
                sqa = sqp.tile([P, cw], F32, tag="sqa")
                sqb = sqp.tile([P, cw], F32, tag="sqb")
                nc.scalar.activation(out=sqa, in_=ksb[:, 0, :], func=AF.Square)
                nc.scalar.activation(out=sqb, in_=ksb[:, 1, :], func=AF.Square)
                nc.vector.tensor_tensor(out=s01, in0=sqa, in1=sqb, op=OP.add)
                sqc = sqp.tile([P, cw], F32, tag="sqa")
                sqd = sqp.tile([P, cw], F32, tag="sqb")
                nc.scalar.activation(out=sqc, in_=ksb[:, 2, :], func=AF.Square)
                nc.scalar.activation(out=sqd, in_=ksb[:, 3, :], func=AF.Square)
                nc.vector.tensor_tensor(out=s01, in0=s01, in1=sqc, op=OP.add)
                nc.vector.tensor_tensor(out=s01, in0=s01, in1=sqd, op=OP.add)
                n_ps = psum_n.tile([1, cw], F32, tag="nps")
                nc.tensor.matmul(n_ps, lhsT=ones_col, rhs=s01, start=True, stop=True)
                # invS = S / sqrt(ss) = 1/sqrt(ss/S^2)
                nsb = sqp.tile([1, cw], F32, tag="nsb")
                nc.scalar.activation(out=nsb, in_=n_ps, func=AF.Sqrt,
                                     scale=1.0 / (S_CONST * S_CONST))
                inv_sb = sqp.tile([1, cw], F32, tag="inv")
                nc.vector.reciprocal(out=inv_sb, in_=nsb)
                invb_ps = psum_bc.tile([P, cw], F32, tag="bc")
                nc.tensor.matmul(invb_ps, lhsT=ones_row, rhs=inv_sb,
                                 start=True, stop=True)
                for kt in range(NT):
                    nc.vector.tensor_tensor(out=ksb[:, kt, :], in0=ksb[:, kt, :],
                                            in1=invb_ps, op=OP.mult)
                for t in range(NT):
                    mm_ps = psum_mm.tile([P, cw], F32, tag="mm")
                    for kt in range(NT):
                        nc.tensor.matmul(mm_ps,
                                         lhsT=embT_sb[:, kt, t * P:(t + 1) * P],
                                         rhs=ksb[:, kt, :],
                                         start=(kt == 0), stop=(kt == NT - 1))
                    nc.scalar.activation(out=out_sb[:, t, c0:c0 + cw], in_=mm_ps,
                                         func=AF.Copy)
                nc.sync.dma_start(out=o1.ap()[:, c0:c0 + cw].rearrange(
                                      "(t p) c -> p t c", p=P),
                                  in_=out_sb[:, :, c0:c0 + cw])
                nc.sync.dma_start(out=o2.ap()[:, c0:c0 + cw].rearrange(
                                      "(t p) c -> p t c", p=P),
                                  in_=out_sb[:, :, c0:c0 + cw])
                # chunk-granular partial argmax, overlapped with later c-tiles
                for ch, (clo, chi) in enumerate(CHUNKS):
                    if ci != chi - 1:
                        continue
                    ch0 = sum(CT[:clo])
                    ch1 = sum(CT[:chi])
                    for t in range(NT):
                        if ci == len(CT) - 1:
                            # mask shard padding before the last chunk's max
                            nc.vector.tensor_tensor(
                                out=out_sb[:, t, CSH - NPAD:CSH],
                                in0=out_sb[:, t, CSH - NPAD:CSH],
                                in1=padmask_sb, op=OP.add)
                        nc.vector.max(out=mx_sb[:, t, ch, :],
                                      in_=out_sb[:, t, ch0:ch1])
                        nc.vector.max_index(out=idx_sb[:, t, ch, :],
                                            in_max=mx_sb[:, t, ch, :],
                                            in_values=out_sb[:, t, ch0:ch1])
                c0 += cw

        # --------- local argmax candidates ---------
        NCH = len(CHUNKS)
        idxf_sb = small.tile([P, NT, NCH, 8], F32)
        candv_sb = small.tile([P, NT, 2], F32)
        candi_sb = small.tile([P, NT, 2], F32)
        bigpos24 = small.tile([P, NCH * 8], F32)
        nc.vector.memset(bigpos24, POS_BIG)
        for t in range(NT):
            nc.vector.tensor_copy(out=idxf_sb[:, t], in_=idx_sb[:, t])
            # global index: + coloff + chunk base
            for ch, (clo, chi) in enumerate(CHUNKS):
                nc.vector.tensor_scalar(out=idxf_sb[:, t, ch, :],
                                        in0=idxf_sb[:, t, ch, :],
                                        scalar1=coloff_sb[:, t:t + 1],
                                        scalar2=float(sum(CT[:clo])),
                                        op0=OP.add, op1=OP.add)
            vals24 = mx_sb[:, t].rearrange("p ch e -> p (ch e)")
            gidx24 = idxf_sb[:, t].rearrange("p ch e -> p (ch e)")
            # penalize the label column so the merged max excludes it
            pen = small.tile([P, NCH * 8], F32, tag="pen")
            nc.vector.tensor_scalar(out=pen, in0=gidx24,
                                    scalar1=labglob_sb[:, t:t + 1], scalar2=-2.0e9,
                                    op0=OP.is_equal, op1=OP.mult)
            valsp = small.tile([P, NCH * 8], F32, tag="valsp")
            nc.vector.tensor_tensor(out=valsp, in0=vals24, in1=pen, op=OP.add)
            ev = small.tile([P, 1], F32, tag="ev")
            nc.vector.reduce_max(out=ev, in_=valsp, axis=mybir.AxisListType.X)
            eqx = small.tile([P, NCH * 8], mybir.dt.int32, tag="eqx")
            nc.vector.tensor_scalar(out=eqx, in0=valsp, scalar1=ev, scalar2=None,
                                    op0=OP.is_equal)
            idxm = small.tile([P, NCH * 8], F32, tag="idxm")
            nc.vector.select(out=idxm, mask=eqx, on_true=gidx24,
                             on_false=bigpos24)
            ei = small.tile([P, 1], F32, tag="ei")
            nc.vector.tensor_reduce(out=ei, in_=idxm, op=OP.min,
                                    axis=mybir.AxisListType.X)
            nc.vector.tensor_copy(out=candv_sb[:, t, 0:1], in_=ev)
            nc.vector.tensor_copy(out=candi_sb[:, t, 0:1], in_=ei)
            # label candidate (only if the label is in this shard)
            nc.vector.select(out=candv_sb[:, t, 1:2], mask=labmask_i[:, t:t + 1],
                             on_true=clabS[:, t:t + 1], on_false=negbig1)
            nc.vector.tensor_copy(out=candi_sb[:, t, 1:2],
                                  in_=labglob_sb[:, t:t + 1])
            # local winner with min-index tie-break
            wv2 = small.tile([P, 1], F32, tag="wv2")
            nc.vector.reduce_max(out=wv2, in_=candv_sb[:, t, :],
                                 axis=mybir.AxisListType.X)
            eq2 = small.tile([P, 2], mybir.dt.int32, tag="eq2")
            nc.vector.tensor_scalar(out=eq2, in0=candv_sb[:, t, :], scalar1=wv2,
                                    scalar2=None, op0=OP.is_equal)
            im2 = small.tile([P, 2], F32, tag="im2")
            nc.vector.select(out=im2, mask=eq2, on_true=candi_sb[:, t, :],
                             on_false=bigpos8[:, 0:2])
            wi2 = small.tile([P, 1], F32, tag="wi2")
            nc.vector.tensor_reduce(out=wi2, in_=im2, op=OP.min,
                                    axis=mybir.AxisListType.X)
            nc.vector.tensor_copy(out=candv_sb[:, t, 0:1], in_=wv2)
            nc.vector.tensor_copy(out=candi_sb[:, t, 0:1], in_=wi2)

        # pack candidates to DRAM: row i -> (val, idx)
        cpack = small.tile([P, NT, 2], F32)
        for t in range(NT):
            nc.vector.tensor_copy(out=cpack[:, t, 0:1], in_=candv_sb[:, t, 0:1])
            nc.vector.tensor_copy(out=cpack[:, t, 1:2], in_=candi_sb[:, t, 0:1])
        nc.sync.dma_start(out=cand_dram.ap(), in_=cpack)

        # --------- cross-core argmax ---------
        nc.gpsimd.collective_compute(
            "AllGather", OP.bypass,
            replica_groups=[list(range(NCORES))],
            ins=[cand_dram.ap().opt()],
            outs=[gath_dram.ap().opt()],
        )
        allc = small.tile([P, NT, NCORES, 2], F32)
        gath_r = gath_dram.ap().rearrange("(c p t) x -> t p c x", p=P, t=NT)
        for t in range(NT):
            nc.sync.dma_start(out=allc[:, t, :, :], in_=gath_r[t])
        amax_sb = small.tile([P, NT], F32)
        f2a_sb = small.tile([P, NT], F32)
        for t in range(NT):
            gv = small.tile([P, 1], F32, tag="gv")
            nc.vector.reduce_max(out=gv, in_=allc[:, t, :, 0],
                                 axis=mybir.AxisListType.X)
            geq = small.tile([P, 8], mybir.dt.int32, tag="geq")
            nc.vector.tensor_scalar(out=geq, in0=allc[:, t, :, 0], scalar1=gv,
                                    scalar2=None, op0=OP.is_equal)
            gim = small.tile([P, 8], F32, tag="gim")
            nc.vector.select(out=gim, mask=geq, on_true=allc[:, t, :, 1],
                             on_false=bigpos8)
            nc.vector.tensor_reduce(out=amax_sb[:, t:t + 1], in_=gim, op=OP.min,
                                    axis=mybir.AxisListType.X)
            # w = gv/S ; cos_max_am = clip2(w*cosg - sqrt(max(1-w^2,0))*sing)
            w = small.tile([P, 1], F32, tag="w")
            nc.vector.tensor_scalar(out=w, in0=gv, scalar1=1.0 / S_CONST,
                                    scalar2=None, op0=OP.mult)
            w2 = small.tile([P, 1], F32, tag="w2")
            nc.vector.tensor_tensor(out=w2, in0=w, in1=w, op=OP.mult)
            nc.vector.tensor_scalar(out=w2, in0=w2, scalar1=-1.0, scalar2=1.0,
                                    op0=OP.mult, op1=OP.add)
            nc.vector.tensor_scalar(out=w2, in0=w2, scalar1=0.0, scalar2=None,
                                    op0=OP.max)
            sw = small.tile([P, 1], F32, tag="sw")
            nc.scalar.activation(out=sw, in_=w2, func=AF.Sqrt)
            cm = small.tile([P, 1], F32, tag="cm")
            tb = small.tile([P, 1], F32, tag="tb")
            nc.vector.tensor_tensor(out=cm, in0=w, in1=cosg_sb[:, t:t + 1],
                                    op=OP.mult)
            nc.vector.tensor_tensor(out=tb, in0=sw, in1=sing_sb[:, t:t + 1],
                                    op=OP.mult)
            nc.vector.tensor_tensor(out=cm, in0=cm, in1=tb, op=OP.subtract)
            nc.vector.tensor_scalar(out=cm, in0=cm, scalar1=COS_LO, scalar2=COS_HI,
                                    op0=OP.max, op1=OP.min)
            nc.vector.tensor_tensor(out=cm, in0=cm, in1=gadd_sb[:, t:t + 1],
                                    op=OP.subtract)
            nc.vector.tensor_scalar(out=f2a_sb[:, t:t + 1], in0=cm, scalar1=S_CONST,
                                    scalar2=None, op0=OP.mult)
        nc.sync.dma_start(out=amax.ap(), in_=amax_sb)
        nc.sync.dma_start(out=fix2a.ap(), in_=f2a_sb)

    nc.compile()
    return nc


_NC_CACHE = None


def _get_nc():
    global _NC_CACHE
    if _NC_CACHE is None:
        _NC_CACHE = build_nc()
    return _NC_CACHE


LAST_RESULT = None  # run_bass_kernel_spmd result (for test harness trace access)


def make_in_maps(embbedings, norms, label, kernel):
    emb = np.ascontiguousarray(np.asarray(embbedings, dtype=np.float32))
    norms = np.asarray(norms, dtype=np.float32).reshape(B)
    label_i = np.asarray(label).astype(np.int64)
    kern = np.asarray(kernel, dtype=np.float32)

    embT = np.ascontiguousarray(emb.T)
    klab = np.ascontiguousarray(kern[:, label_i])
    labglob = label_i.astype(np.float32)

    in_maps = []
    for c in range(NCORES):
        lo = c * CSH
        hi = min(lo + CSH, C)
        ks = kern[:, lo:hi]
        if hi - lo < CSH:
            ks = np.concatenate(
                [ks, np.ones((K, CSH - (hi - lo)), np.float32)], axis=1)
        pm = np.zeros(NPAD, np.float32)
        if hi - lo < CSH:
            pm[:] = PAD_MASK_VAL
        in_maps.append({
            "embT": embT,
            "ksh": np.ascontiguousarray(ks),
            "klab": klab,
            "normsv": norms,
            "labloc": (label_i - lo).astype(np.float32),
            "labmask": ((label_i >= lo) & (label_i < hi)).astype(np.float32),
            "labglob": labglob,
            "coloff": np.full(B, lo, np.float32),
            "padmask": pm,
        })
    return in_maps, label_i


def _unperm(v):
    # device writes [p, t] contiguously; row i = t*128 + p
    return np.asarray(v).reshape(P, NT).T.reshape(B)


def assemble(outs, label_i, label_dtype):
    o1 = np.concatenate([outs[c]["o1"] for c in range(NCORES)], axis=1)[:, :C]
    o2 = np.concatenate([outs[c]["o2"] for c in range(NCORES)], axis=1)[:, :C]
    am = _unperm(outs[0]["amax"]).astype(np.int64)
    rows = np.arange(B)
    o1[rows, label_i] = _unperm(outs[0]["fix1"])
    o2[rows, label_i] = _unperm(outs[0]["fix2l"])
    o2[rows, am] = _unperm(outs[0]["fix2a"])
    marg = _unperm(outs[0]["marg"]).reshape(B, 1).astype(np.float32)
    idx_dtype = np.int64 if np.dtype(label_dtype) == np.int64 else np.int32
    return (o1, o2, am.astype(idx_dtype), marg)


def kernel(embbedings, norms, label, kernel):
    global LAST_RESULT
    in_maps, label_i = make_in_maps(embbedings, norms, label, kernel)
    nc = _get_nc()
    trace = os.environ.get("KERNEL_TRACE", "0") == "1"
    res = run_bass_kernel_spmd(nc, in_maps, core_ids=list(range(NCORES)),
                               trace=trace)
    LAST_RESULT = res
    return assemble(res.results, label_i, np.asarray(label).dtype)
